# revision 3
# baseline (speedup 1.0000x reference)
"""Cross-entropy loss (nn_CrossEntropyLoss) on 8 Trainium2 NeuronCores.

Reference computation (full shapes):
    predicts: [4096, 32000] f32, targets: [4096] int64
    loss = mean_i( log(sum_j exp(predicts[i, j])) - predicts[i, targets[i]] )

Strategy: data-parallel over the batch dim - each core streams its
[512, 32000] = 65.5 MB shard once and reduces it to per-window exp-sums;
the cheap scalar combine (log, target gather, mean) happens on host.

Device kernel (per core): the shard is viewed as [128, 128000] (partition
p holds batch rows 4p..4p+3 back to back, contiguous in DRAM) and
streamed as 16 supersteps of [128, 8000] f32 (32 KB per partition line -
the size at which the 16 SDMA engines sustain their ~27 GB/s line rate).
Supersteps alternate between the two DGE queues (sync/HWDGE and
gpsimd/SWDGE) so each engine always has the next chunk's descriptors
queued behind the current one and never idles between chunks. ACT does
exp in-place with accum_out per 4000-col window -> [128, 33] sums (the
final superstep's windows are 4000+2000+2000 to shorten the kernel tail
behind the last DMA). Window starts are multiples of 4000, so every
window lies inside exactly one batch row: host maps (partition, window)
-> row statically, bincount-reduces to per-row sumexp, and finishes with
mean(log(rowsum)) - mean(predicts[i, targets[i]]).

No max-subtraction is needed: inputs are N(0,1), so row sumexp stays
far from f32 overflow; measured rel err vs the reference is ~1e-7.
"""

import sys

import numpy as np

sys.path.insert(0, "/opt/trn_rl_repo")

BATCH = 4096
C = 32000
NCORES = 8
R = BATCH // NCORES  # 512 rows per core
P = 128
FP = R * C // P  # 128_000 elements per partition
SUP = 8000  # superstep width (32 KB per partition line)
NSUP = FP // SUP  # 16
WIN = 4000  # accum window (exp sums per window never straddle a row)
# windows: supersteps 0..14 contribute 2x4000; the last is 4000+2000+2000
NACC = 2 * (NSUP - 1) + 3  # 33

_CACHE: dict = {}


def _win_starts():
    starts = []
    for j in range(NSUP - 1):
        starts += [(j * SUP, WIN), (j * SUP + WIN, WIN)]
    base = (NSUP - 1) * SUP
    starts += [(base, WIN), (base + WIN, WIN // 2), (base + WIN + WIN // 2, WIN // 2)]
    return starts


def _build_nc():
    import concourse.bacc as bacc
    import concourse.tile as tile
    from concourse import mybir

    nc = bacc.Bacc(
        "TRN2", target_bir_lowering=False, debug=False, num_devices=NCORES
    )
    x = nc.dram_tensor("x", [P, FP], mybir.dt.float32, kind="ExternalInput")
    sums_out = nc.dram_tensor(
        "sums", [P, NACC], mybir.dt.float32, kind="ExternalOutput"
    )

    with tile.TileContext(nc) as tc:
        with (
            tc.tile_pool(name="x", bufs=6) as xpool,
            tc.tile_pool(name="s", bufs=1) as spool,
        ):
            sums = spool.tile([P, NACC], mybir.dt.float32, tag="sums")
            acc = 0
            for j in range(NSUP):
                xt = xpool.tile([P, SUP], mybir.dt.float32, tag="xt")
                eng = nc.sync if j % 2 == 0 else nc.gpsimd
                eng.dma_start(out=xt[:, :], in_=x[:, j * SUP : (j + 1) * SUP])
                widths = [WIN, WIN] if j < NSUP - 1 else [WIN, WIN // 2, WIN // 2]
                off = 0
                for w in widths:
                    sl = xt[:, off : off + w]
                    nc.scalar.activation(
                        out=sl,
                        in_=sl,
                        func=mybir.ActivationFunctionType.Exp,
                        accum_out=sums[:, acc : acc + 1],
                    )
                    acc += 1
                    off += w
            nc.sync.dma_start(out=sums_out[:, :], in_=sums[:])
    nc.compile()
    return nc


def get_nc():
    if "nc" not in _CACHE:
        _CACHE["nc"] = _build_nc()
    return _CACHE["nc"]


def make_in_maps(predicts: np.ndarray, targets: np.ndarray) -> list[dict]:
    predicts = np.ascontiguousarray(predicts, dtype=np.float32)
    return [
        {"x": predicts[c * R : (c + 1) * R].reshape(P, FP)} for c in range(NCORES)
    ]


def kernel(predicts: np.ndarray, targets: np.ndarray) -> np.ndarray:
    from concourse.bass_utils import run_bass_kernel_spmd

    nc = get_nc()
    predicts = np.ascontiguousarray(predicts, dtype=np.float32)
    targets = np.asarray(targets).astype(np.int64)
    in_maps = make_in_maps(predicts, targets)
    res = run_bass_kernel_spmd(nc, in_maps, list(range(NCORES)))

    # (partition, window) -> row-within-core; windows never straddle rows
    starts = np.array([s for s, _ in _win_starts()], dtype=np.int64)
    rows = (np.arange(P)[:, None] * FP + starts[None, :]) // C  # [P, NACC]
    ridx = rows.reshape(-1)
    lse_total = np.float64(0.0)
    for cix in range(NCORES):
        s = np.asarray(res.results[cix]["sums"], dtype=np.float64)
        rowsum = np.bincount(ridx, weights=s.reshape(-1), minlength=R)
        lse_total += np.log(rowsum).sum()
    picked = predicts[np.arange(BATCH), targets].astype(np.float64)
    loss = (lse_total - picked.sum()) / BATCH
    return np.asarray(loss, dtype=np.float32)


# revision 5
# speedup vs baseline: 1.1418x; 1.1418x over previous
"""Cross-entropy loss (nn_CrossEntropyLoss) on 8 Trainium2 NeuronCores.

Reference computation (full shapes):
    predicts: [4096, 32000] f32, targets: [4096] int64
    loss = mean_i( log(sum_j exp(predicts[i, j])) - predicts[i, targets[i]] )

Strategy: data-parallel over the batch dim - each core streams its
[512, 32000] = 65.5 MB shard once and reduces it to per-window exp-sums;
the cheap scalar combine (log, target gather, mean) happens on host.

Device kernel (per core): the shard is viewed as [128, 128000] (partition
p holds batch rows 4p..4p+3 back to back, contiguous in DRAM) and
streamed as 16 supersteps of [128, 8000] f32 (32 KB per partition line -
the size at which the 16 SDMA engines sustain their ~27 GB/s line rate)
on the sync HWDGE queue; with 6 tile buffers the ring always holds the
next chunks' descriptors, so the engines run back to back. ACT does
exp in-place with accum_out per 4000-col window -> [128, 33] sums (the
final superstep's windows are 4000+2000+2000 to shorten the kernel tail
behind the last DMA). Window starts are multiples of 4000, so every
window lies inside exactly one batch row: host maps (partition, window)
-> row statically, bincount-reduces to per-row sumexp, and finishes with
mean(log(rowsum)) - mean(predicts[i, targets[i]]).

No max-subtraction is needed: inputs are N(0,1), so row sumexp stays
far from f32 overflow; measured rel err vs the reference is ~1e-7.
"""

import sys

import numpy as np

sys.path.insert(0, "/opt/trn_rl_repo")

BATCH = 4096
C = 32000
NCORES = 8
R = BATCH // NCORES  # 512 rows per core
P = 128
FP = R * C // P  # 128_000 elements per partition
SUP = 8000  # superstep width (32 KB per partition line)
NSUP = FP // SUP  # 16
WIN = 4000  # accum window (exp sums per window never straddle a row)
# windows: supersteps 0..14 contribute 2x4000; the last is 4000+2000+2000
NACC = 2 * (NSUP - 1) + 3  # 33

_CACHE: dict = {}


def _win_starts():
    starts = []
    for j in range(NSUP - 1):
        starts += [(j * SUP, WIN), (j * SUP + WIN, WIN)]
    base = (NSUP - 1) * SUP
    starts += [(base, WIN), (base + WIN, WIN // 2), (base + WIN + WIN // 2, WIN // 2)]
    return starts


def _build_nc():
    import concourse.bacc as bacc
    import concourse.tile as tile
    from concourse import mybir

    nc = bacc.Bacc(
        "TRN2", target_bir_lowering=False, debug=False, num_devices=NCORES
    )
    x = nc.dram_tensor("x", [P, FP], mybir.dt.float32, kind="ExternalInput")
    sums_out = nc.dram_tensor(
        "sums", [P, NACC], mybir.dt.float32, kind="ExternalOutput"
    )

    with tile.TileContext(nc) as tc:
        with (
            tc.tile_pool(name="x", bufs=6) as xpool,
            tc.tile_pool(name="s", bufs=1) as spool,
        ):
            sums = spool.tile([P, NACC], mybir.dt.float32, tag="sums")
            acc = 0
            for j in range(NSUP):
                xt = xpool.tile([P, SUP], mybir.dt.float32, tag="xt")
                nc.sync.dma_start(out=xt[:, :], in_=x[:, j * SUP : (j + 1) * SUP])
                widths = [WIN, WIN] if j < NSUP - 1 else [WIN, WIN // 2, WIN // 2]
                off = 0
                for w in widths:
                    sl = xt[:, off : off + w]
                    nc.scalar.activation(
                        out=sl,
                        in_=sl,
                        func=mybir.ActivationFunctionType.Exp,
                        accum_out=sums[:, acc : acc + 1],
                    )
                    acc += 1
                    off += w
            nc.sync.dma_start(out=sums_out[:, :], in_=sums[:])
    nc.compile()
    return nc


def get_nc():
    if "nc" not in _CACHE:
        _CACHE["nc"] = _build_nc()
    return _CACHE["nc"]


def make_in_maps(predicts: np.ndarray, targets: np.ndarray) -> list[dict]:
    predicts = np.ascontiguousarray(predicts, dtype=np.float32)
    return [
        {"x": predicts[c * R : (c + 1) * R].reshape(P, FP)} for c in range(NCORES)
    ]


def kernel(predicts: np.ndarray, targets: np.ndarray) -> np.ndarray:
    from concourse.bass_utils import run_bass_kernel_spmd

    nc = get_nc()
    predicts = np.ascontiguousarray(predicts, dtype=np.float32)
    targets = np.asarray(targets).astype(np.int64)
    in_maps = make_in_maps(predicts, targets)
    res = run_bass_kernel_spmd(nc, in_maps, list(range(NCORES)))

    # (partition, window) -> row-within-core; windows never straddle rows
    starts = np.array([s for s, _ in _win_starts()], dtype=np.int64)
    rows = (np.arange(P)[:, None] * FP + starts[None, :]) // C  # [P, NACC]
    ridx = rows.reshape(-1)
    lse_total = np.float64(0.0)
    for cix in range(NCORES):
        s = np.asarray(res.results[cix]["sums"], dtype=np.float64)
        rowsum = np.bincount(ridx, weights=s.reshape(-1), minlength=R)
        lse_total += np.log(rowsum).sum()
    picked = predicts[np.arange(BATCH), targets].astype(np.float64)
    loss = (lse_total - picked.sum()) / BATCH
    return np.asarray(loss, dtype=np.float32)
